# revision 1
# baseline (speedup 1.0000x reference)
"""Llama GQA attention (B=2, S=2048, HID=2048, H=32, HKV=8, DH=64) on 8 TRN2 cores.

Sharding: tensor-parallel over heads. Core c owns q heads [4c, 4c+4) and kv
head c. One SPMD NEFF per run:
  1. Q/K/V projections in transposed layout (fp32r matmuls at full PE rate),
     interleaved per 512-token block with the attention that consumes them,
  2. RoPE via a signed-permutation matmul + DVE combines,
  3. causal flash attention with scores kept transposed [k, q] so the PV
     matmul needs no on-chip transposes; softmax sums come from a ones-row
     appended to V; no max subtraction (scores are small for this problem);
     diagonal tiles are column-sliced so only the visible staircase is
     computed and a single [128,128] mask handles the mixed subtile,
  4. per-batch AllGather of the normalized context (ctx^T, [256, 2048]),
  5. column-sharded o_proj: each core produces out^T rows [256c, 256c+256).
Host pre-transposes inputs and assembles the 8 output slices.
"""
import sys

sys.path.insert(0, "/opt/trn_rl_repo")

import numpy as np

B, S, HID = 2, 2048, 2048
H, HKV, DH = 32, 8, 64
NC = 8
T = B * S
HPC = H // NC            # q heads per core (4)
CPC = HPC * DH           # ctx dims per core (256)
TB = 512                 # token block
KC = 128                 # k chunk
QBS = S // TB            # 4 q blocks per batch
SB_KC = S // KC          # 16 k chunks per batch
HCH = HID // 128         # 16 hid chunks
SCALE = DH ** -0.5
NEG = -1.0e30


def _build(causal: bool, reps: int = 1, phases: str = "all", bf16: bool = False):
    """phases: 'all' | 'proj' (projections+RoPE only) | 'noop' (skip o_proj+AG).
    bf16: stream hidden/weights/context I/O in bfloat16 (attention stays f32r)."""
    import concourse.mybir as mybir
    import concourse.tile as tile
    from concourse import bacc
    from concourse.masks import make_identity

    F32 = mybir.dt.float32
    F32R = mybir.dt.float32r
    BF16 = mybir.dt.bfloat16
    IOD = BF16 if bf16 else F32R
    EXPF = mybir.ActivationFunctionType.Exp
    ADD = mybir.AluOpType.add
    MUL = mybir.AluOpType.mult

    nc = bacc.Bacc("TRN2", target_bir_lowering=False, debug=False, num_devices=NC)

    hT = nc.dram_tensor("hT", [HID, T], IOD, kind="ExternalInput")
    wqT = nc.dram_tensor("wqT", [HID, CPC], IOD, kind="ExternalInput")
    wkvT = nc.dram_tensor("wkvT", [HID, 2 * DH], IOD, kind="ExternalInput")
    woT = nc.dram_tensor("woT", [H * DH, CPC], IOD, kind="ExternalInput")
    cosT = nc.dram_tensor("cosT", [DH, T], F32, kind="ExternalInput")
    sinT = nc.dram_tensor("sinT", [DH, T], F32, kind="ExternalInput")
    rotp = nc.dram_tensor("rotp", [DH, DH], F32R, kind="ExternalInput")
    if causal:
        maskd = nc.dram_tensor("maskd", [128, 128], F32, kind="ExternalInput")
    else:
        maskg = nc.dram_tensor("maskg", [S, S], F32, kind="ExternalInput")
    outT = nc.dram_tensor("outT", [CPC, T], F32, kind="ExternalOutput")

    with tile.TileContext(nc) as tc:
        with tc.tile_pool(name="const", bufs=1) as cpool, \
             tc.tile_pool(name="big", bufs=1) as big, \
             tc.tile_pool(name="stream", bufs=3) as stream, \
             tc.tile_pool(name="rope", bufs=2) as rope, \
             tc.tile_pool(name="attn", bufs=3) as attn, \
             tc.tile_pool(name="psM", bufs=1, space="PSUM") as psM, \
             tc.tile_pool(name="psS", bufs=3, space="PSUM") as psS, \
             tc.tile_pool(name="psC", bufs=1, space="PSUM") as psC, \
             tc.tile_pool(name="dram", bufs=1, space="DRAM") as dram:

            # ---- persistent SBUF ----
            wq_sb = cpool.tile([128, HCH, CPC], IOD)
            nc.sync.dma_start(wq_sb[:], wqT[:].rearrange("(o p) m -> p o m", p=128))
            wkv_sb = cpool.tile([128, HCH, 2 * DH], IOD)
            nc.sync.dma_start(wkv_sb[:], wkvT[:].rearrange("(o p) m -> p o m", p=128))
            wo_sb = cpool.tile([128, HCH, CPC], IOD)
            nc.sync.dma_start(wo_sb[:], woT[:].rearrange("(o p) m -> p o m", p=128))
            cos_sb = cpool.tile([DH, T], F32)
            nc.sync.dma_start(cos_sb[:], cosT[:])
            sin_sb = cpool.tile([DH, T], F32)
            nc.sync.dma_start(sin_sb[:], sinT[:])
            rot_sb = cpool.tile([DH, DH], F32R)
            nc.sync.dma_start(rot_sb[:], rotp[:])
            if causal:
                mk_sb = cpool.tile([128, 128], F32)
                nc.sync.dma_start(mk_sb[:], maskd[:])
            onesc_f = cpool.tile([128, SB_KC, 1], F32)
            nc.any.memset(onesc_f[:], 1.0)
            ident = cpool.tile([DH, DH], F32)
            make_identity(nc, ident)

            # ---- per-batch big activation buffers ----
            qT_sb = [[big.tile([128, S], F32R, tag=f"qT{b}{hp}", name=f"qT{b}{hp}")
                      for hp in range(2)] for b in range(B)]
            kT_sb = [big.tile([128, S], F32R, tag=f"kT{b}", name=f"kT{b}")
                     for b in range(B)]
            v_sb = [big.tile([128, SB_KC, DH + 1], F32R, tag=f"v{b}", name=f"v{b}")
                    for b in range(B)]
            for b in range(B):
                nc.vector.tensor_copy(v_sb[b][:, :, DH:DH + 1], onesc_f[:])

            ag_in = [[dram.tile([CPC, S], IOD, name=f"agi{b}_{r}")
                      for b in range(B)] for r in range(reps)]
            ag_out = [[dram.tile([H * DH, S], IOD, name=f"ago{b}_{r}",
                                 addr_space="Shared") for b in range(B)]
                      for r in range(reps)]

            def proj_block(b, qb, rep):
                tb = b * QBS + qb
                gs = slice(tb * TB, (tb + 1) * TB)      # global token slice
                ls = slice(qb * TB, (qb + 1) * TB)      # slice within batch
                pq = [psM.tile([128, TB], F32, tag=f"mm{hp}", name=f"pq{hp}_{tb}_{rep}")
                      for hp in range(2)]
                pkv = psM.tile([128, TB], F32, tag="mmkv")
                for cc in range(HCH):
                    h_sb = stream.tile([128, TB], IOD, tag="h")
                    nc.sync.dma_start(h_sb[:], hT[cc * 128:(cc + 1) * 128, gs])
                    for hp in range(2):
                        nc.tensor.matmul(pq[hp][:],
                                         wq_sb[:, cc, hp * 128:(hp + 1) * 128],
                                         h_sb[:], start=(cc == 0),
                                         stop=(cc == HCH - 1))
                    nc.tensor.matmul(pkv[:], wkv_sb[:, cc, :], h_sb[:],
                                     start=(cc == 0), stop=(cc == HCH - 1))
                # RoPE on q heads, one 64-row half at a time
                for h in range(HPC):
                    hp, hh = h // 2, 64 * (h % 2)
                    src = pq[hp][hh:hh + 64, :]
                    qraw = rope.tile([DH, TB], F32R, tag="raw")
                    nc.vector.tensor_copy(qraw[:], src)
                    qcos = rope.tile([DH, TB], F32, tag="cos")
                    nc.vector.tensor_tensor(qcos[:], qraw[:].bitcast(F32),
                                            cos_sb[:, gs], MUL)
                    rps = psS.tile([DH, TB], F32, tag="s")
                    nc.tensor.matmul(rps[:], rot_sb[:], qraw[:], start=True, stop=True)
                    qsin = rope.tile([DH, TB], F32, tag="sin")
                    nc.vector.tensor_tensor(qsin[:], rps[:], sin_sb[:, gs], MUL)
                    if hh == 0:
                        nc.vector.tensor_tensor(qT_sb[b][hp][0:64, ls],
                                                qcos[:], qsin[:], ADD)
                    else:
                        qfin = rope.tile([DH, TB], F32R, tag="fin")
                        nc.vector.tensor_tensor(qfin[:], qcos[:], qsin[:], ADD)
                        nc.sync.dma_start(qT_sb[b][hp][64:128, ls], qfin[:])
                # RoPE on K (kv psum rows 64:128; V in rows 0:64)
                ksrc = pkv[64:128, :]
                kraw = rope.tile([DH, TB], F32R, tag="raw")
                nc.vector.tensor_copy(kraw[:], ksrc)
                kcos = rope.tile([DH, TB], F32, tag="cos")
                nc.vector.tensor_tensor(kcos[:], kraw[:].bitcast(F32),
                                        cos_sb[:, gs], MUL)
                krps = psS.tile([DH, TB], F32, tag="s")
                nc.tensor.matmul(krps[:], rot_sb[:], kraw[:], start=True, stop=True)
                ksin = rope.tile([DH, TB], F32, tag="sin")
                nc.vector.tensor_tensor(ksin[:], krps[:], sin_sb[:, gs], MUL)
                nc.vector.tensor_tensor(kT_sb[b][0:64, ls], kcos[:], ksin[:], ADD)
                # duplicate K^T rows for base-64 matmuls
                nc.sync.dma_start(kT_sb[b][64:128, ls], kT_sb[b][0:64, ls])
                # V: psum rows 0:64 -> transpose into [128, 64] chunks
                vraw = rope.tile([DH, TB], F32, tag="vraw")
                nc.vector.tensor_copy(vraw[:], pkv[0:64, :])
                for i in range(TB // KC):
                    vtp = psS.tile([128, DH], F32, tag="s")
                    nc.tensor.transpose(vtp[:], vraw[:, i * KC:(i + 1) * KC], ident[:])
                    nc.vector.tensor_copy(v_sb[b][:, qb * (TB // KC) + i, 0:DH], vtp[:])

            def attn_block(b, qb, rep):
                for hp in range(2):
                    kcs = list(range(4 * qb + 4)) if causal else list(range(SB_KC))
                    # two heads of the pair run their K=64 score matmuls in
                    # different PE row-groups (bases 0/64) concurrently
                    ctxp = [psC.tile([DH + 1, TB], F32, tag=f"ctx{x}",
                                     name=f"ctx{x}_{rep}_{b}_{qb}_{hp}")
                            for x in range(2)]
                    for i, kc in enumerate(kcs):
                        c0 = 128 * (kc - 4 * qb) if (causal and kc >= 4 * qb) else 0
                        qsl = slice(qb * TB + c0, (qb + 1) * TB)
                        sps = [psS.tile([128, TB], F32, tag="s",
                                        name=f"s{x}_{rep}_{b}_{qb}_{hp}_{kc}")
                               for x in range(2)]
                        for x, hh in enumerate((0, 64)):
                            nc.tensor.matmul(
                                sps[x][:, c0:TB],
                                kT_sb[b][hh:hh + 64, kc * KC:(kc + 1) * KC],
                                qT_sb[b][hp][hh:hh + 64, qsl],
                                start=True, stop=True)
                        for x in range(2):
                            if causal:
                                if kc >= 4 * qb:
                                    nc.vector.tensor_tensor(
                                        sps[x][:, c0:c0 + 128],
                                        sps[x][:, c0:c0 + 128], mk_sb[:], ADD)
                            else:
                                mg = attn.tile([128, TB], F32, tag="mg")
                                nc.sync.dma_start(
                                    mg[:], maskg[kc * KC:(kc + 1) * KC,
                                                 qb * TB:(qb + 1) * TB])
                                nc.vector.tensor_tensor(sps[x][:], sps[x][:],
                                                        mg[:], ADD)
                            p_sb = attn.tile([128, TB], F32R, tag="p")
                            nc.scalar.activation(p_sb[:, c0:TB], sps[x][:, c0:TB],
                                                 EXPF, scale=SCALE)
                            nc.tensor.matmul(ctxp[x][:, c0:TB], v_sb[b][:, kc, :],
                                             p_sb[:, c0:TB], start=(i == 0),
                                             stop=(i == len(kcs) - 1),
                                             skip_group_check=True)
                    for x in range(2):
                        h = 2 * hp + x
                        rc = attn.tile([1, TB], F32R, tag="rc")
                        with nc.allow_low_precision(reason="f32r rounding ~1e-4"):
                            nc.vector.reciprocal(rc[:], ctxp[x][DH:DH + 1, :])
                        rb = attn.tile([DH, TB], F32R, tag="rb")
                        nc.gpsimd.partition_broadcast(rb[:], rc[:])
                        ctxn = attn.tile([DH, TB], IOD, tag="ctxn")
                        nc.vector.tensor_tensor(ctxn[:], ctxp[x][0:DH, :],
                                                rb[:].bitcast(mybir.dt.float32), MUL)
                        nc.sync.dma_start(
                            ag_in[rep][b][h * DH:(h + 1) * DH,
                                          qb * TB:(qb + 1) * TB],
                            ctxn[:])

            for rep in range(reps):
                for b in range(B):
                    for qb in range(QBS):
                        proj_block(b, qb, rep)
                        if phases != "proj":
                            attn_block(b, qb, rep)
                    if phases == "proj":
                        # flush accumulators so the phase is observable
                        for hp in range(2):
                            dbg = stream.tile([128, TB], F32, tag="o")
                            nc.vector.tensor_copy(dbg[:], qT_sb[b][hp][:, 0:TB].bitcast(F32))
                            nc.sync.dma_start(outT[hp * 128:(hp + 1) * 128,
                                                   b * S:b * S + TB], dbg[:])
                        continue
                    if phases == "noop":
                        continue
                    nc.gpsimd.collective_compute(
                        "AllGather", mybir.AluOpType.bypass,
                        replica_groups=[list(range(NC))],
                        ins=[ag_in[rep][b].opt()], outs=[ag_out[rep][b].opt()])

                if phases != "all":
                    continue
                # ---- o_proj: out^T slice [CPC, T] ----
                for b in range(B):
                    for qb in range(QBS):
                        ls = slice(qb * TB, (qb + 1) * TB)
                        gs = slice((b * QBS + qb) * TB, (b * QBS + qb + 1) * TB)
                        po = [psM.tile([128, TB], F32, tag=f"mm{o}",
                                       name=f"po{o}_{b}_{qb}_{rep}")
                              for o in range(2)]
                        for cc in range(HCH):
                            c_sb = stream.tile([128, TB], IOD, tag="c")
                            nc.sync.dma_start(
                                c_sb[:], ag_out[rep][b][cc * 128:(cc + 1) * 128, ls])
                            for o in range(2):
                                nc.tensor.matmul(po[o][:],
                                                 wo_sb[:, cc, o * 128:(o + 1) * 128],
                                                 c_sb[:], start=(cc == 0),
                                                 stop=(cc == HCH - 1))
                        for o in range(2):
                            o_sb = stream.tile([128, TB], F32, tag="o")
                            nc.vector.tensor_copy(o_sb[:], po[o][:])
                            nc.sync.dma_start(outT[o * 128:(o + 1) * 128, gs],
                                              o_sb[:])

    nc.compile()
    return nc


def _host_inputs(hidden_states, cos, sin, attention_mask, Wq, Wk, Wv, Wo, causal,
                 bf16=False):
    hT = np.ascontiguousarray(hidden_states.reshape(T, HID).T)
    cosT = np.ascontiguousarray(cos.reshape(T, DH).T)
    sinT = np.ascontiguousarray(sin.reshape(T, DH).T)
    # rot_half as a signed permutation: rot[d] = -x[d+32] (d<32), +x[d-32] (d>=32)
    p64 = np.zeros((DH, DH), np.float32)
    for m in range(32):
        p64[m + 32, m] = -1.0
        p64[m, m + 32] = 1.0
    WqT = np.ascontiguousarray(Wq.T)      # [HID, H*DH]
    WkT = np.ascontiguousarray(Wk.T)      # [HID, HKV*DH]
    WvT = np.ascontiguousarray(Wv.T)
    WoT = np.ascontiguousarray(Wo.T)      # [H*DH, HID]

    if bf16:
        import ml_dtypes
        bf = ml_dtypes.bfloat16
        hT = hT.astype(bf)
        WqT, WkT, WvT, WoT = (w.astype(bf) for w in (WqT, WkT, WvT, WoT))
    ins = []
    for c in range(NC):
        d = {
            "hT": hT,
            "wqT": np.ascontiguousarray(WqT[:, c * CPC:(c + 1) * CPC]),
            "wkvT": np.ascontiguousarray(
                np.concatenate([WvT[:, c * DH:(c + 1) * DH],
                                WkT[:, c * DH:(c + 1) * DH]], axis=1)),
            "woT": np.ascontiguousarray(WoT[:, c * CPC:(c + 1) * CPC]),
            "cosT": cosT, "sinT": sinT, "rotp": p64,
        }
        if causal:
            i = np.arange(128, dtype=np.float32)[:, None]
            cc = np.arange(128, dtype=np.float32)[None, :]
            d["maskd"] = np.where(cc < i, NEG, 0.0).astype(np.float32)
        else:
            m = attention_mask[0, 0].astype(np.float32)
            d["maskg"] = np.ascontiguousarray(m.T) * np.float32(1.0 / SCALE)
        ins.append(d)
    return ins


def _is_causal(attention_mask):
    if attention_mask.shape != (1, 1, S, S):
        return False
    m = attention_mask[0, 0]
    neg = np.finfo(np.float32).min
    tril = np.tril(np.ones((S, S), dtype=bool))
    expect = np.where(tril, np.float32(0.0), np.float32(neg))
    return np.array_equal(m, expect)


_CACHE = {}


BF16_IO = False


def _get_nc(causal, reps=1, phases="all", bf16=None):
    if bf16 is None:
        bf16 = BF16_IO
    key = (causal, reps, phases, bf16)
    if key not in _CACHE:
        _CACHE[key] = _build(causal, reps, phases, bf16)
    return _CACHE[key]


def kernel(**inputs) -> np.ndarray:
    from concourse.bass_utils import run_bass_kernel_spmd

    hidden_states = np.asarray(inputs["hidden_states"], np.float32)
    cos = np.asarray(inputs["cos"], np.float32)
    sin = np.asarray(inputs["sin"], np.float32)
    attention_mask = np.asarray(inputs["attention_mask"], np.float32)
    Wq = np.asarray(inputs["Wq"], np.float32)
    Wk = np.asarray(inputs["Wk"], np.float32)
    Wv = np.asarray(inputs["Wv"], np.float32)
    Wo = np.asarray(inputs["Wo"], np.float32)

    causal = _is_causal(attention_mask)
    nc = _get_nc(causal)
    ins = _host_inputs(hidden_states, cos, sin, attention_mask,
                       Wq, Wk, Wv, Wo, causal, bf16=BF16_IO)
    res = run_bass_kernel_spmd(nc, ins, core_ids=list(range(NC)))
    outT = np.concatenate([res.results[c]["outT"] for c in range(NC)], axis=0)
    return np.ascontiguousarray(outT.T).reshape(B, S, HID)

